# revision 41
# baseline (speedup 1.0000x reference)
"""Trainium2 Bass kernel for nn_BCErrorCNN (dense_cnn).

Network (per sample, input [17, 9]):
  Conv1D(128, k=3, relu) -> [15, 128]   (position 14 dead: never consumed)
  LocallyConnected1D(128, k=3, relu) -> [13, 128]  (position 12 dead)
  MaxPool1D(2) -> [6, 128]
  LocallyConnected1D(128, k=3, relu) -> [4, 128]
  GlobalAvgPool -> [128]; Dense(100, relu); Dense(1, sigmoid)

Sharding: pure data parallelism, batch 32768 -> 8 cores x 4096.
~149.5us HW exec vs the 186.4us fp32r baseline; rel err ~2.4e-4.

Fully fp16 datapath (PSUM accumulation stays fp32):
  - X transposed to [feature, batch] by the DMA XBAR straight out of DRAM
    (dma_start(transpose=True), 16-bit only): one [512,128]->[128,512]
    instruction per TA/TB per tile on the otherwise-idle SP engine, so no
    PE transposes, no fp32 input load, and the ACT engine keeps its whole
    budget for PSUM evacuation.
  - conv reads TA/TB directly with zero-padded weights at legal 32-aligned
    base partitions (tile_position rule: K<=32 -> any 32-multiple,
    K<=64 -> {0,64}, else 0) -- no SBUF->SBUF strip DMAs.  Paired,
    row-band-disjoint conv matmuls share a [128,1024] PSUM tile and a
    single strided evacuation, and partially overlap in the PE array.
  - conv pairs + lc1 triples interleaved in PE issue order, plus a
    half-stage software pipeline: each tile's last three lc1 triples, its
    lc2+mean and its dense tail run under the NEXT tile's conv pairs, so
    every serial evac chain has microseconds of PE cover.
  - lc1 evac fused with maxpool: ACT relu-evacs the even position, DVE
    scalar_tensor_tensor computes max(odd+b, relu(even)) which equals
    relu(max(even+b, odd+b)) since relu(x) >= 0.
  - lc2 evac fused with the global-average fold (zero-bias path): two ACT
    relu evacs + two DVE scalar_tensor_tensor accumulations leave the two
    pairwise sums; d1 (wd1 pre-scaled by 1/4) consumes them as two
    accumulating matmuls, so there is no serial mean-reduce chain at all.
  - weights ride the gpsimd software DGE so the hwdge DMA semaphores that
    matmuls wait on count only input transposes; zero biases (the common
    case) are specialized away entirely.
  - per-tile sigmoid straight out of PSUM + per-tile output DMA.
"""

import functools

import numpy as np

# ---- constants (hardcoded per problem spec) --------------------------------
N_CORES = 8
B_FULL = 32768
BC = B_FULL // N_CORES  # per-core batch
NB = 512                # batch tile (columns per matmul)
NT = BC // NB           # batch tiles per core
LIN, CIN, F = 17, 9, 128
FEAT = LIN * CIN        # 153
NPOS = 14               # conv positions actually needed (0..13)
NL1 = 12                # lc1 positions needed (0..11)
NPOOL = 6
NL2 = 4
ND1 = 100

# Conv position p contracts feature rows 9p..9p+26.  TA holds features
# 0..127 on partitions 0..127, TB holds features 25..152.  The matmul
# base-partition rule constrains tile_position[0] by contraction size K:
# K<=32 -> {0,32,64,96}; K<=64 -> {0,64}; else 0.  q0 below is the
# partition where wc row 0 sits (TA: 9p; TB: 9p-25), base is the chosen
# 32-aligned start, K = q0 + 27 - base.
CONV_GEO = [
    # (src, base, q0, K): contraction K is padded to >=64 where the PE
    # would otherwise run small-K matmuls at ~1.5 cyc/col; the extra rows
    # are zeros in the weights and harmless extra terms from TA/TB.
    (0, 0, 0, 64), (0, 0, 9, 36), (0, 0, 18, 64), (0, 0, 27, 64),
    (0, 0, 36, 64), (0, 0, 45, 72), (0, 0, 54, 81), (0, 0, 63, 90),
    (0, 0, 72, 99), (0, 64, 81, 44), (0, 64, 90, 53), (0, 0, 99, 126),
    (1, 0, 83, 110), (1, 64, 92, 55),
]
# Issue order: consecutive matmuls sit in disjoint PE row ranges where
# possible so the systolic array overlaps them.
CONV_ORDER = [0, 8, 1, 9, 2, 10, 5, 11, 4, 12, 3, 13, 6, 7]


def _build_program(nt=NT, bias_flags=(True, True, True, True)):
    conv_bias_zero, lc1_bias_zero, lc2_bias_zero, d1_bias_zero = bias_flags
    import concourse.tile as tile
    from concourse import bacc, mybir

    F32 = mybir.dt.float32
    F16 = mybir.dt.float16
    AF = mybir.ActivationFunctionType
    ALU = mybir.AluOpType

    bc = nt * NB
    nc = bacc.Bacc("TRN2", target_bir_lowering=False, debug=False,
                   num_devices=N_CORES)

    x = nc.dram_tensor("x", [bc * FEAT], F16, kind="ExternalInput").ap()
    wcp = nc.dram_tensor("wcp", [128, NPOS * F], F16, kind="ExternalInput").ap()
    w1 = nc.dram_tensor("w1", [128, NL1 * 3 * F], F16, kind="ExternalInput").ap()
    w2 = nc.dram_tensor("w2", [128, NL2 * 3 * F], F16, kind="ExternalInput").ap()
    wd1 = nc.dram_tensor("wd1", [F, 128], F16, kind="ExternalInput").ap()
    wd2 = nc.dram_tensor("wd2", [ND1, 128], F16, kind="ExternalInput").ap()
    cb = nc.dram_tensor("cb", [F, 1], F32, kind="ExternalInput").ap()
    b1 = nc.dram_tensor("b1", [F, NL1], F32, kind="ExternalInput").ap()
    b2 = nc.dram_tensor("b2", [F, NL2], F32, kind="ExternalInput").ap()
    db = nc.dram_tensor("db", [ND1, 1], F32, kind="ExternalInput").ap()
    y = nc.dram_tensor("y", [bc], F32, kind="ExternalOutput").ap()

    with tile.TileContext(nc) as tc:
        with (
            tc.tile_pool(name="const", bufs=1) as cpool,
            tc.tile_pool(name="t", bufs=2) as tpool,
            tc.tile_pool(name="h", bufs=2) as hpool,
            tc.tile_pool(name="eo", bufs=3) as epool,
            tc.tile_pool(name="m", bufs=2) as mpool,
            tc.tile_pool(name="s2", bufs=2) as s2pool,
            tc.tile_pool(name="ss", bufs=2) as sspool,
            tc.tile_pool(name="s3", bufs=2) as s3pool,
            tc.tile_pool(name="ys", bufs=2) as ypool,
            tc.tile_pool(name="psC", bufs=2, space="PSUM") as psC,
            tc.tile_pool(name="psL", bufs=4, space="PSUM") as psL,
        ):
            def load_T(jt):
                # DMA-XBAR transpose: DRAM [batch, feat] -> SBUF [feat, batch]
                # One [512,128]->[128,512] instruction per target, on sync
                # (tile 0's TB goes on scalar so TA/TB land in parallel).
                TA = tpool.tile([128, NB], F16, tag="TA", name=f"TA{jt}")
                TB = tpool.tile([128, NB], F16, tag="TB", name=f"TB{jt}")
                b0 = jt * NB * FEAT
                srcA = x[b0:b0 + 1].copy()
                srcA.ap = srcA.ap[:0] + [[FEAT, NB], [1, 128]]
                nc.sync.dma_start(TA[:], srcA, transpose=True)
                srcB = x[b0 + 25:b0 + 26].copy()
                srcB.ap = srcB.ap[:0] + [[FEAT, NB], [1, 128]]
                eng = nc.scalar if jt == 0 else nc.sync
                eng.dma_start(TB[:], srcB, transpose=True)
                return TA, TB

            T_pre = {0: load_T(0)}

            # ---- weights (+ any nonzero biases) on the gpsimd software
            # DGE so the hwdge (sync/scalar) DMA semaphores that conv/lc
            # matmuls wait on count only transposes.
            wcpt = cpool.tile([128, NPOS * F], F16)
            nc.gpsimd.dma_start(wcpt[:, :11 * F], wcp[:, :11 * F])
            cbt = b1t = b2t = dbt = None
            if not conv_bias_zero:
                cbt = cpool.tile([F, 1], F32)
                nc.gpsimd.dma_start(cbt[:], cb[:])
            if not lc1_bias_zero:
                b1t = cpool.tile([F, NL1], F32)
                nc.gpsimd.dma_start(b1t[:], b1[:])
            if not lc2_bias_zero:
                b2t = cpool.tile([F, NL2], F32)
                nc.gpsimd.dma_start(b2t[:], b2[:])
            if not d1_bias_zero:
                dbt = cpool.tile([ND1, 1], F32)
                nc.gpsimd.dma_start(dbt[:], db[:])
            nc.gpsimd.dma_start(wcpt[:, 11 * F:], wcp[:, 11 * F:])
            w1t = cpool.tile([128, NL1 * 3 * F], F16)
            nc.gpsimd.dma_start(w1t[:, :NL1 * 3 * F // 2],
                                w1[:, :NL1 * 3 * F // 2])
            nc.gpsimd.dma_start(w1t[:, NL1 * 3 * F // 2:],
                                w1[:, NL1 * 3 * F // 2:])
            w2t = cpool.tile([128, NL2 * 3 * F], F16)
            nc.gpsimd.dma_start(w2t[:], w2[:])
            wd1t = cpool.tile([F, 128], F16)
            nc.gpsimd.dma_start(wd1t[:], wd1[:])
            wd2t = cpool.tile([ND1, 128], F16)
            nc.gpsimd.dma_start(wd2t[:], wd2[:])

            state = {}

            def conv_pair(a, TA, TB, H, it):
                # two row-band-disjoint positions share one [128,1024] psum
                # tile and a single strided evacuation into H.  PSUM half 0
                # always holds the lower position so the H stride stays
                # positive regardless of issue order.
                pi, pj = CONV_ORDER[2 * a], CONV_ORDER[2 * a + 1]
                lo, hi = min(pi, pj), max(pi, pj)
                ps = psC.tile([128, 2 * NB], F32, tag="C", name=f"pC{it}_{a}")
                for p in (pi, pj):
                    src, base, q0, K = CONV_GEO[p]
                    T = TA if src == 0 else TB
                    half = 0 if p == lo else 1
                    nc.tensor.matmul(
                        ps[:, half * NB:(half + 1) * NB],
                        wcpt[base:base + K, p * F:(p + 1) * F],
                        T[base:base + K, :],
                        start=True, stop=True, tile_position=(base, 0))
                hdst = H[:, lo * NB:lo * NB + 1].copy()
                hdst.ap = hdst.ap[:1] + [[(hi - lo) * NB, 2], [1, NB]]
                if a % 2 == 0:
                    if conv_bias_zero:
                        nc.scalar.activation(hdst, ps[:], AF.Relu)
                    else:
                        nc.scalar.activation(hdst, ps[:], AF.Relu,
                                             bias=cbt[:])
                elif conv_bias_zero:
                    nc.vector.tensor_scalar_max(hdst, ps[:], 0.0)
                else:
                    nc.vector.tensor_scalar(hdst, ps[:], cbt[:], 0.0,
                                            op0=ALU.add, op1=ALU.max)

            def lc1_triple(l, H, M, it):
                ps = psL.tile([128, NB], F32, tag="L", name=f"pL{it}_{l}")
                for k in range(3):
                    nc.tensor.matmul(
                        ps[:], w1t[:, (l * 3 + k) * F:(l * 3 + k + 1) * F],
                        H[:, (l + k) * NB:(l + k + 1) * NB],
                        start=(k == 0), stop=(k == 2))
                t = l // 2
                if l % 2 == 0:
                    EO = epool.tile([128, NB], F16, tag="E",
                                    name=f"E{it}_{t}")
                    if lc1_bias_zero:
                        nc.scalar.activation(EO[:], ps[:], AF.Relu)
                    else:
                        nc.scalar.activation(EO[:], ps[:], AF.Relu,
                                             bias=b1t[:, l:l + 1])
                    state[("eo", t)] = EO
                else:
                    EO = state.pop(("eo", t))
                    if lc1_bias_zero:
                        nc.vector.scalar_tensor_tensor(
                            M[:, t * NB:(t + 1) * NB], ps[:], 0.0,
                            EO[:], op0=ALU.max, op1=ALU.max)
                    else:
                        nc.vector.scalar_tensor_tensor(
                            M[:, t * NB:(t + 1) * NB], ps[:],
                            b1t[:, l:l + 1], EO[:],
                            op0=ALU.add, op1=ALU.max)

            def lc2_mean(M, S2, it):
                # lc2 + global-average fold.  Zero-bias path fuses the mean
                # into the evacuations: A_l = relu(ps_l) for l=0,1 (ACT),
                # then DVE scalar_tensor_tensor accumulates relu(ps_{l+2})
                # on top, and one DVE add produces Ssum.
                pss = []
                for l in range(NL2):
                    ps = psL.tile([128, NB], F32, tag="L", name=f"pT{it}_{l}")
                    for k in range(3):
                        nc.tensor.matmul(
                            ps[:], w2t[:, (l * 3 + k) * F:(l * 3 + k + 1) * F],
                            M[:, (l + k) * NB:(l + k + 1) * NB],
                            start=(k == 0), stop=(k == 2))
                    pss.append(ps)
                if lc2_bias_zero:
                    A = sspool.tile([128, 2 * NB], F16, tag="A", name=f"A{it}")
                    for h in range(2):
                        nc.scalar.activation(A[:, h * NB:(h + 1) * NB],
                                             pss[h][:], AF.Relu)
                        nc.vector.scalar_tensor_tensor(
                            S2[:, h * NB:(h + 1) * NB], pss[h + 2][:], 0.0,
                            A[:, h * NB:(h + 1) * NB],
                            op0=ALU.max, op1=ALU.add)
                    state[("s2half", it)] = S2
                else:
                    for l in range(NL2):
                        sdst = S2[:, l * NB:(l + 1) * NB]
                        if l % 2 == 0:
                            nc.scalar.activation(sdst, pss[l][:], AF.Relu,
                                                 bias=b2t[:, l:l + 1])
                        else:
                            nc.vector.tensor_scalar(
                                sdst, pss[l][:], b2t[:, l:l + 1], 0.0,
                                op0=ALU.add, op1=ALU.max)
                    A = sspool.tile([128, 2 * NB], F16, tag="A", name=f"A{it}")
                    nc.gpsimd.tensor_tensor(A[:, 0:NB], S2[:, 0:NB],
                                            S2[:, NB:2 * NB], op=ALU.add)
                    nc.gpsimd.tensor_tensor(A[:, NB:2 * NB],
                                            S2[:, 2 * NB:3 * NB],
                                            S2[:, 3 * NB:4 * NB], op=ALU.add)
                    state[("s2half", it)] = A

            def tail_a(it, h):
                # d1 as two accumulating matmuls over the S2 half-sums
                # (mean folded via wd1 pre-scale): no serial mean chain,
                # each half is consumed as soon as its evac lands.
                if h == 0:
                    pD = psL.tile([128, NB], F32, tag="L", name=f"pD{it}")
                    state[("pd", it)] = pD
                else:
                    pD = state[("pd", it)]
                Shalf = state[("s2half", it)] if h == 0 else \
                    state.pop(("s2half", it))
                nc.tensor.matmul(pD[:, :], wd1t[:],
                                 Shalf[:, h * NB:(h + 1) * NB],
                                 start=(h == 0), stop=(h == 1))

            def tail_b(it):
                pD = state.pop(("pd", it))
                S3 = s3pool.tile([ND1, NB], F16, tag="S3", name=f"S3{it}")
                if d1_bias_zero:
                    nc.scalar.activation(S3[:], pD[0:ND1, :], AF.Relu)
                else:
                    nc.scalar.activation(S3[:], pD[0:ND1, :], AF.Relu,
                                         bias=dbt[:])
                # d2 reuses row 0 of the same bank; the WAR on d1's region is
                # already ordered by the S3 evac that d2 depends on.
                nc.tensor.matmul(pD[:, :], wd2t[:], S3[:],
                                 start=True, stop=True)
                Ys = ypool.tile([1, NB], F32, tag="Y", name=f"Y{it}")
                nc.scalar.activation(Ys[:], pD[0:1, :], AF.Sigmoid)
                nc.sync.dma_start(y[it * NB:(it + 1) * NB], Ys[0:1, :])

            for it in range(nt):
                TA, TB = T_pre.pop(it)
                if it + 1 < nt:
                    T_pre[it + 1] = load_T(it + 1)
                H = hpool.tile([128, NPOS * NB], F16, tag="H", name=f"H{it}")
                M = mpool.tile([128, NPOOL * NB], F16, tag="M", name=f"M{it}")
                S2 = s2pool.tile([128, NL2 * NB], F16, tag="S2",
                                 name=f"S2{it}")

                # conv pairs + lc1 triples interleaved; the previous tile's
                # serial tail chain (mean->d1->relu->d2) hides behind convs.
                # Half-stage software pipeline: the previous tile's last
                # three lc1 triples, its lc2+mean, and its dense tail all
                # run under this tile's conv pairs, so every serial evac
                # chain has multiple us of PE work for cover.
                conv_pair(0, TA, TB, H, it)    # p0, p8
                conv_pair(1, TA, TB, H, it)    # p1, p9
                if it > 0:
                    Hp, Mp, S2p = state.pop(("hms", it - 1))
                    for l in (5, 6, 7):
                        lc1_triple(l, Hp, Mp, it - 1)
                conv_pair(2, TA, TB, H, it)    # p2, p10
                if it > 0:
                    lc2_mean(Mp, S2p, it - 1)
                conv_pair(3, TA, TB, H, it)    # p5, p11
                if it > 0:
                    tail_a(it - 1, 0)
                lc1_triple(0, H, M, it)
                if it > 0:
                    tail_a(it - 1, 1)
                lc1_triple(8, H, M, it)
                if it > 0:
                    tail_b(it - 1)
                conv_pair(4, TA, TB, H, it)    # p4, p12
                lc1_triple(9, H, M, it)
                conv_pair(5, TA, TB, H, it)    # p3, p13
                lc1_triple(10, H, M, it)
                conv_pair(6, TA, TB, H, it)    # p6, p7
                lc1_triple(11, H, M, it)
                for l in (1, 2, 3, 4):
                    lc1_triple(l, H, M, it)
                state[("hms", it)] = (H, M, S2)

            it = nt - 1
            Hp, Mp, S2p = state.pop(("hms", it))
            for l in (5, 6, 7):
                lc1_triple(l, Hp, Mp, it)
            lc2_mean(Mp, S2p, it)
            tail_a(it, 0)
            tail_a(it, 1)
            tail_b(it)

    nc.compile()
    return nc


@functools.lru_cache(maxsize=4)
def _get_program(nt, bias_flags=(True, True, True, True)):
    return _build_program(nt, bias_flags)


def _prep_in_maps(inputs, conv_w, conv_b, lc1_w, lc1_b, lc2_w, lc2_b,
                  d1_w, d1_b, d2_w, nt=NT, n_cores=N_CORES):
    bc = nt * NB
    bias_flags = (not np.any(conv_b), not np.any(lc1_b[:NL1]),
                  not np.any(lc2_b), not np.any(d1_b))
    f16, f32 = np.float16, np.float32

    wc = np.asarray(conv_w, dtype=f32).reshape(27, F)
    wcp_np = np.zeros((128, NPOS * F), dtype=f16)
    for p, (_, _, q0, _) in enumerate(CONV_GEO):
        wcp_np[q0:q0 + 27, p * F:(p + 1) * F] = wc
    w1_np = np.ascontiguousarray(
        np.asarray(lc1_w[:NL1], dtype=f32).reshape(NL1, 3, F, F)
        .transpose(2, 0, 1, 3).reshape(128, NL1 * 3 * F)).astype(f16)
    w2_np = np.ascontiguousarray(
        np.asarray(lc2_w, dtype=f32).reshape(NL2, 3, F, F)
        .transpose(2, 0, 1, 3).reshape(128, NL2 * 3 * F)).astype(f16)
    wd1_np = np.zeros((F, 128), dtype=f16)
    wd1_np[:, :ND1] = (np.asarray(d1_w, dtype=f32) * 0.25).astype(f16)
    wd2_np = np.zeros((ND1, 128), dtype=f16)
    wd2_np[:, 0] = np.asarray(d2_w, dtype=f32).reshape(ND1).astype(f16)
    cb_np = np.ascontiguousarray(conv_b.reshape(F, 1), dtype=f32)
    b1_np = np.ascontiguousarray(np.asarray(lc1_b[:NL1], dtype=f32).T)
    b2_np = np.ascontiguousarray(np.asarray(lc2_b, dtype=f32).T)
    db_np = np.ascontiguousarray(d1_b.reshape(ND1, 1), dtype=f32)
    shared = dict(wcp=wcp_np, w1=w1_np, w2=w2_np, wd1=wd1_np, wd2=wd2_np,
                  cb=cb_np, b1=b1_np, b2=b2_np, db=db_np)

    x16 = np.asarray(inputs, dtype=f16).reshape(n_cores, bc * FEAT)
    in_maps = [dict(shared, x=x16[c]) for c in range(n_cores)]
    return in_maps, bias_flags


def kernel(inputs, conv_w, conv_b, lc1_w, lc1_b, lc2_w, lc2_b,
           d1_w, d1_b, d2_w):
    from concourse.bass_utils import run_bass_kernel_spmd

    in_maps, bias_flags = _prep_in_maps(
        inputs, conv_w, conv_b, lc1_w, lc1_b, lc2_w, lc2_b, d1_w, d1_b, d2_w)
    nc = _get_program(NT, bias_flags)
    res = run_bass_kernel_spmd(nc, in_maps, list(range(N_CORES)))
    out = np.concatenate([res.results[c]["y"] for c in range(N_CORES)])
    return out.reshape(B_FULL, 1).astype(np.float32)


# revision 43
# speedup vs baseline: 1.0214x; 1.0214x over previous
"""Trainium2 Bass kernel for nn_BCErrorCNN (dense_cnn).

Network (per sample, input [17, 9]):
  Conv1D(128, k=3, relu) -> [15, 128]   (position 14 dead: never consumed)
  LocallyConnected1D(128, k=3, relu) -> [13, 128]  (position 12 dead)
  MaxPool1D(2) -> [6, 128]
  LocallyConnected1D(128, k=3, relu) -> [4, 128]
  GlobalAvgPool -> [128]; Dense(100, relu); Dense(1, sigmoid)

Sharding: pure data parallelism, batch 32768 -> 8 cores x 4096.
~149.5us HW exec vs the 186.4us fp32r baseline; rel err ~2.4e-4.

Fully fp16 datapath (PSUM accumulation stays fp32):
  - X transposed to [feature, batch] by the DMA XBAR straight out of DRAM
    (dma_start(transpose=True), 16-bit only): one [512,128]->[128,512]
    instruction per TA/TB per tile on the otherwise-idle SP engine, so no
    PE transposes, no fp32 input load, and the ACT engine keeps its whole
    budget for PSUM evacuation.
  - conv reads TA/TB directly with zero-padded weights at legal 32-aligned
    base partitions (tile_position rule: K<=32 -> any 32-multiple,
    K<=64 -> {0,64}, else 0) -- no SBUF->SBUF strip DMAs.  Paired,
    row-band-disjoint conv matmuls share a [128,1024] PSUM tile and a
    single strided evacuation, and partially overlap in the PE array.
  - conv pairs + lc1 triples interleaved in PE issue order, plus a
    half-stage software pipeline: each tile's last three lc1 triples, its
    lc2+mean and its dense tail run under the NEXT tile's conv pairs, so
    every serial evac chain has microseconds of PE cover.
  - lc1 evac fused with maxpool: ACT relu-evacs the even position, DVE
    scalar_tensor_tensor computes max(odd+b, relu(even)) which equals
    relu(max(even+b, odd+b)) since relu(x) >= 0.
  - lc2 evac fused with the global-average fold (zero-bias path): two ACT
    relu evacs + two DVE scalar_tensor_tensor accumulations leave the two
    pairwise sums; d1 (wd1 pre-scaled by 1/4) consumes them as two
    accumulating matmuls, so there is no serial mean-reduce chain at all.
  - weights ride the gpsimd software DGE so the hwdge DMA semaphores that
    matmuls wait on count only input transposes; zero biases (the common
    case) are specialized away entirely.
  - per-tile sigmoid straight out of PSUM + per-tile output DMA.
"""

import functools

import numpy as np

# ---- constants (hardcoded per problem spec) --------------------------------
N_CORES = 8
B_FULL = 32768
BC = B_FULL // N_CORES  # per-core batch
NB = 512                # batch tile (columns per matmul)
NT = BC // NB           # batch tiles per core
LIN, CIN, F = 17, 9, 128
FEAT = LIN * CIN        # 153
NPOS = 14               # conv positions actually needed (0..13)
NL1 = 12                # lc1 positions needed (0..11)
NPOOL = 6
NL2 = 4
ND1 = 100

# Conv position p contracts feature rows 9p..9p+26.  TA holds features
# 0..127 on partitions 0..127, TB holds features 25..152.  The matmul
# base-partition rule constrains tile_position[0] by contraction size K:
# K<=32 -> {0,32,64,96}; K<=64 -> {0,64}; else 0.  q0 below is the
# partition where wc row 0 sits (TA: 9p; TB: 9p-25), base is the chosen
# 32-aligned start, K = q0 + 27 - base.
CONV_GEO = [
    # (src, base, q0, K): contraction K is padded to >=64 where the PE
    # would otherwise run small-K matmuls at ~1.5 cyc/col; the extra rows
    # are zeros in the weights and harmless extra terms from TA/TB.
    (0, 0, 0, 64), (0, 0, 9, 36), (0, 0, 18, 64), (0, 0, 27, 64),
    (0, 0, 36, 64), (0, 0, 45, 72), (0, 0, 54, 81), (0, 0, 63, 90),
    (0, 64, 72, 64), (0, 64, 81, 44), (0, 64, 90, 53), (0, 64, 99, 64),
    (1, 64, 83, 64), (1, 64, 92, 55),
]
# Issue order: consecutive matmuls sit in disjoint PE row ranges where
# possible so the systolic array overlaps them.
CONV_ORDER = [0, 8, 1, 9, 2, 10, 5, 11, 4, 12, 3, 13, 6, 7]


def _build_program(nt=NT, bias_flags=(True, True, True, True)):
    conv_bias_zero, lc1_bias_zero, lc2_bias_zero, d1_bias_zero = bias_flags
    import concourse.tile as tile
    from concourse import bacc, mybir

    F32 = mybir.dt.float32
    F16 = mybir.dt.float16
    AF = mybir.ActivationFunctionType
    ALU = mybir.AluOpType

    bc = nt * NB
    nc = bacc.Bacc("TRN2", target_bir_lowering=False, debug=False,
                   num_devices=N_CORES)

    x = nc.dram_tensor("x", [bc * FEAT], F16, kind="ExternalInput").ap()
    wcp = nc.dram_tensor("wcp", [128, NPOS * F], F16, kind="ExternalInput").ap()
    w1 = nc.dram_tensor("w1", [128, NL1 * 3 * F], F16, kind="ExternalInput").ap()
    w2 = nc.dram_tensor("w2", [128, NL2 * 3 * F], F16, kind="ExternalInput").ap()
    wd1 = nc.dram_tensor("wd1", [F, 128], F16, kind="ExternalInput").ap()
    wd2 = nc.dram_tensor("wd2", [ND1, 128], F16, kind="ExternalInput").ap()
    cb = nc.dram_tensor("cb", [F, 1], F32, kind="ExternalInput").ap()
    b1 = nc.dram_tensor("b1", [F, NL1], F32, kind="ExternalInput").ap()
    b2 = nc.dram_tensor("b2", [F, NL2], F32, kind="ExternalInput").ap()
    db = nc.dram_tensor("db", [ND1, 1], F32, kind="ExternalInput").ap()
    y = nc.dram_tensor("y", [bc], F32, kind="ExternalOutput").ap()

    with tile.TileContext(nc) as tc:
        with (
            tc.tile_pool(name="const", bufs=1) as cpool,
            tc.tile_pool(name="t", bufs=2) as tpool,
            tc.tile_pool(name="h", bufs=2) as hpool,
            tc.tile_pool(name="eo", bufs=3) as epool,
            tc.tile_pool(name="m", bufs=2) as mpool,
            tc.tile_pool(name="s2", bufs=2) as s2pool,
            tc.tile_pool(name="ss", bufs=2) as sspool,
            tc.tile_pool(name="s3", bufs=2) as s3pool,
            tc.tile_pool(name="ys", bufs=2) as ypool,
            tc.tile_pool(name="psC", bufs=2, space="PSUM") as psC,
            tc.tile_pool(name="psL", bufs=4, space="PSUM") as psL,
        ):
            def load_T(jt):
                # DMA-XBAR transpose: DRAM [batch, feat] -> SBUF [feat, batch]
                # One [512,128]->[128,512] instruction per target, on sync
                # (tile 0's TB goes on scalar so TA/TB land in parallel).
                TA = tpool.tile([128, NB], F16, tag="TA", name=f"TA{jt}")
                TB = tpool.tile([128, NB], F16, tag="TB", name=f"TB{jt}")
                b0 = jt * NB * FEAT
                srcA = x[b0:b0 + 1].copy()
                srcA.ap = srcA.ap[:0] + [[FEAT, NB], [1, 128]]
                nc.sync.dma_start(TA[:], srcA, transpose=True)
                srcB = x[b0 + 25:b0 + 26].copy()
                srcB.ap = srcB.ap[:0] + [[FEAT, NB], [1, 128]]
                eng = nc.scalar if jt == 0 else nc.sync
                eng.dma_start(TB[:], srcB, transpose=True)
                return TA, TB

            T_pre = {0: load_T(0)}

            # ---- weights (+ any nonzero biases) on the gpsimd software
            # DGE so the hwdge (sync/scalar) DMA semaphores that conv/lc
            # matmuls wait on count only transposes.
            wcpt = cpool.tile([128, NPOS * F], F16)
            nc.gpsimd.dma_start(wcpt[:], wcp[:])
            cbt = b1t = b2t = dbt = None
            if not conv_bias_zero:
                cbt = cpool.tile([F, 1], F32)
                nc.gpsimd.dma_start(cbt[:], cb[:])
            if not lc1_bias_zero:
                b1t = cpool.tile([F, NL1], F32)
                nc.gpsimd.dma_start(b1t[:], b1[:])
            if not lc2_bias_zero:
                b2t = cpool.tile([F, NL2], F32)
                nc.gpsimd.dma_start(b2t[:], b2[:])
            if not d1_bias_zero:
                dbt = cpool.tile([ND1, 1], F32)
                nc.gpsimd.dma_start(dbt[:], db[:])
            w1t = cpool.tile([128, NL1 * 3 * F], F16)
            nc.gpsimd.dma_start(w1t[:, :NL1 * 3 * F // 2],
                                w1[:, :NL1 * 3 * F // 2])
            nc.gpsimd.dma_start(w1t[:, NL1 * 3 * F // 2:],
                                w1[:, NL1 * 3 * F // 2:])
            w2t = cpool.tile([128, NL2 * 3 * F], F16)
            nc.gpsimd.dma_start(w2t[:], w2[:])
            wd1t = cpool.tile([F, 128], F16)
            nc.gpsimd.dma_start(wd1t[:], wd1[:])
            wd2t = cpool.tile([ND1, 128], F16)
            nc.gpsimd.dma_start(wd2t[:], wd2[:])

            state = {}

            def conv_pair(a, TA, TB, H, it):
                # two row-band-disjoint positions share one [128,1024] psum
                # tile and a single strided evacuation into H.  PSUM half 0
                # always holds the lower position so the H stride stays
                # positive regardless of issue order.
                pi, pj = CONV_ORDER[2 * a], CONV_ORDER[2 * a + 1]
                lo, hi = min(pi, pj), max(pi, pj)
                ps = psC.tile([128, 2 * NB], F32, tag="C", name=f"pC{it}_{a}")
                for p in (pi, pj):
                    src, base, q0, K = CONV_GEO[p]
                    T = TA if src == 0 else TB
                    half = 0 if p == lo else 1
                    nc.tensor.matmul(
                        ps[:, half * NB:(half + 1) * NB],
                        wcpt[base:base + K, p * F:(p + 1) * F],
                        T[base:base + K, :],
                        start=True, stop=True, tile_position=(base, 0))
                hdst = H[:, lo * NB:lo * NB + 1].copy()
                hdst.ap = hdst.ap[:1] + [[(hi - lo) * NB, 2], [1, NB]]
                if a % 2 == 0:
                    if conv_bias_zero:
                        nc.scalar.activation(hdst, ps[:], AF.Relu)
                    else:
                        nc.scalar.activation(hdst, ps[:], AF.Relu,
                                             bias=cbt[:])
                elif conv_bias_zero:
                    nc.vector.tensor_scalar_max(hdst, ps[:], 0.0)
                else:
                    nc.vector.tensor_scalar(hdst, ps[:], cbt[:], 0.0,
                                            op0=ALU.add, op1=ALU.max)

            def lc1_triple(l, H, M, it):
                ps = psL.tile([128, NB], F32, tag="L", name=f"pL{it}_{l}")
                for k in range(3):
                    nc.tensor.matmul(
                        ps[:], w1t[:, (l * 3 + k) * F:(l * 3 + k + 1) * F],
                        H[:, (l + k) * NB:(l + k + 1) * NB],
                        start=(k == 0), stop=(k == 2))
                t = l // 2
                if l % 2 == 0:
                    EO = epool.tile([128, NB], F16, tag="E",
                                    name=f"E{it}_{t}")
                    if lc1_bias_zero:
                        nc.scalar.activation(EO[:], ps[:], AF.Relu)
                    else:
                        nc.scalar.activation(EO[:], ps[:], AF.Relu,
                                             bias=b1t[:, l:l + 1])
                    state[("eo", t)] = EO
                else:
                    EO = state.pop(("eo", t))
                    if lc1_bias_zero:
                        nc.vector.scalar_tensor_tensor(
                            M[:, t * NB:(t + 1) * NB], ps[:], 0.0,
                            EO[:], op0=ALU.max, op1=ALU.max)
                    else:
                        nc.vector.scalar_tensor_tensor(
                            M[:, t * NB:(t + 1) * NB], ps[:],
                            b1t[:, l:l + 1], EO[:],
                            op0=ALU.add, op1=ALU.max)

            def lc2_mean(M, S2, it):
                # lc2 + global-average fold.  Zero-bias path fuses the mean
                # into the evacuations: A_l = relu(ps_l) for l=0,1 (ACT),
                # then DVE scalar_tensor_tensor accumulates relu(ps_{l+2})
                # on top, and one DVE add produces Ssum.
                pss = []
                for l in range(NL2):
                    ps = psL.tile([128, NB], F32, tag="L", name=f"pT{it}_{l}")
                    for k in range(3):
                        nc.tensor.matmul(
                            ps[:], w2t[:, (l * 3 + k) * F:(l * 3 + k + 1) * F],
                            M[:, (l + k) * NB:(l + k + 1) * NB],
                            start=(k == 0), stop=(k == 2))
                    pss.append(ps)
                if lc2_bias_zero:
                    A = sspool.tile([128, 2 * NB], F16, tag="A", name=f"A{it}")
                    for h in range(2):
                        nc.scalar.activation(A[:, h * NB:(h + 1) * NB],
                                             pss[h][:], AF.Relu)
                        nc.vector.scalar_tensor_tensor(
                            S2[:, h * NB:(h + 1) * NB], pss[h + 2][:], 0.0,
                            A[:, h * NB:(h + 1) * NB],
                            op0=ALU.max, op1=ALU.add)
                    state[("s2half", it)] = S2
                else:
                    for l in range(NL2):
                        sdst = S2[:, l * NB:(l + 1) * NB]
                        if l % 2 == 0:
                            nc.scalar.activation(sdst, pss[l][:], AF.Relu,
                                                 bias=b2t[:, l:l + 1])
                        else:
                            nc.vector.tensor_scalar(
                                sdst, pss[l][:], b2t[:, l:l + 1], 0.0,
                                op0=ALU.add, op1=ALU.max)
                    A = sspool.tile([128, 2 * NB], F16, tag="A", name=f"A{it}")
                    nc.gpsimd.tensor_tensor(A[:, 0:NB], S2[:, 0:NB],
                                            S2[:, NB:2 * NB], op=ALU.add)
                    nc.gpsimd.tensor_tensor(A[:, NB:2 * NB],
                                            S2[:, 2 * NB:3 * NB],
                                            S2[:, 3 * NB:4 * NB], op=ALU.add)
                    state[("s2half", it)] = A

            def tail_a(it, h):
                # d1 as two accumulating matmuls over the S2 half-sums
                # (mean folded via wd1 pre-scale): no serial mean chain,
                # each half is consumed as soon as its evac lands.
                if h == 0:
                    pD = psL.tile([128, NB], F32, tag="L", name=f"pD{it}")
                    state[("pd", it)] = pD
                else:
                    pD = state[("pd", it)]
                Shalf = state[("s2half", it)] if h == 0 else \
                    state.pop(("s2half", it))
                nc.tensor.matmul(pD[:, :], wd1t[:],
                                 Shalf[:, h * NB:(h + 1) * NB],
                                 start=(h == 0), stop=(h == 1))

            def tail_b(it):
                pD = state.pop(("pd", it))
                S3 = s3pool.tile([ND1, NB], F16, tag="S3", name=f"S3{it}")
                if d1_bias_zero:
                    nc.scalar.activation(S3[:], pD[0:ND1, :], AF.Relu)
                else:
                    nc.scalar.activation(S3[:], pD[0:ND1, :], AF.Relu,
                                         bias=dbt[:])
                # d2 reuses row 0 of the same bank; the WAR on d1's region is
                # already ordered by the S3 evac that d2 depends on.
                nc.tensor.matmul(pD[:, :], wd2t[:], S3[:],
                                 start=True, stop=True)
                Ys = ypool.tile([1, NB], F32, tag="Y", name=f"Y{it}")
                nc.scalar.activation(Ys[:], pD[0:1, :], AF.Sigmoid)
                nc.sync.dma_start(y[it * NB:(it + 1) * NB], Ys[0:1, :])

            for it in range(nt):
                TA, TB = T_pre.pop(it)
                if it + 1 < nt:
                    T_pre[it + 1] = load_T(it + 1)
                H = hpool.tile([128, NPOS * NB], F16, tag="H", name=f"H{it}")
                M = mpool.tile([128, NPOOL * NB], F16, tag="M", name=f"M{it}")
                S2 = s2pool.tile([128, NL2 * NB], F16, tag="S2",
                                 name=f"S2{it}")

                # conv pairs + lc1 triples interleaved; the previous tile's
                # serial tail chain (mean->d1->relu->d2) hides behind convs.
                # Half-stage software pipeline: the previous tile's last
                # three lc1 triples, its lc2+mean, and its dense tail all
                # run under this tile's conv pairs, so every serial evac
                # chain has multiple us of PE work for cover.
                conv_pair(0, TA, TB, H, it)    # p0, p8
                conv_pair(1, TA, TB, H, it)    # p1, p9
                if it > 0:
                    Hp, Mp, S2p = state.pop(("hms", it - 1))
                    for l in (5, 6, 7):
                        lc1_triple(l, Hp, Mp, it - 1)
                conv_pair(2, TA, TB, H, it)    # p2, p10
                if it > 0:
                    lc2_mean(Mp, S2p, it - 1)
                conv_pair(3, TA, TB, H, it)    # p5, p11
                if it > 0:
                    tail_a(it - 1, 0)
                lc1_triple(0, H, M, it)
                if it > 0:
                    tail_a(it - 1, 1)
                lc1_triple(8, H, M, it)
                if it > 0:
                    tail_b(it - 1)
                conv_pair(4, TA, TB, H, it)    # p4, p12
                lc1_triple(9, H, M, it)
                conv_pair(5, TA, TB, H, it)    # p3, p13
                lc1_triple(10, H, M, it)
                conv_pair(6, TA, TB, H, it)    # p6, p7
                lc1_triple(11, H, M, it)
                for l in (1, 2, 3, 4):
                    lc1_triple(l, H, M, it)
                state[("hms", it)] = (H, M, S2)

            it = nt - 1
            Hp, Mp, S2p = state.pop(("hms", it))
            for l in (5, 6, 7):
                lc1_triple(l, Hp, Mp, it)
            lc2_mean(Mp, S2p, it)
            tail_a(it, 0)
            tail_a(it, 1)
            tail_b(it)

    nc.compile()
    return nc


@functools.lru_cache(maxsize=4)
def _get_program(nt, bias_flags=(True, True, True, True)):
    return _build_program(nt, bias_flags)


def _prep_in_maps(inputs, conv_w, conv_b, lc1_w, lc1_b, lc2_w, lc2_b,
                  d1_w, d1_b, d2_w, nt=NT, n_cores=N_CORES):
    bc = nt * NB
    bias_flags = (not np.any(conv_b), not np.any(lc1_b[:NL1]),
                  not np.any(lc2_b), not np.any(d1_b))
    f16, f32 = np.float16, np.float32

    wc = np.asarray(conv_w, dtype=f32).reshape(27, F)
    wcp_np = np.zeros((128, NPOS * F), dtype=f16)
    for p, (_, _, q0, _) in enumerate(CONV_GEO):
        wcp_np[q0:q0 + 27, p * F:(p + 1) * F] = wc
    w1_np = np.ascontiguousarray(
        np.asarray(lc1_w[:NL1], dtype=f32).reshape(NL1, 3, F, F)
        .transpose(2, 0, 1, 3).reshape(128, NL1 * 3 * F)).astype(f16)
    w2_np = np.ascontiguousarray(
        np.asarray(lc2_w, dtype=f32).reshape(NL2, 3, F, F)
        .transpose(2, 0, 1, 3).reshape(128, NL2 * 3 * F)).astype(f16)
    wd1_np = np.zeros((F, 128), dtype=f16)
    wd1_np[:, :ND1] = (np.asarray(d1_w, dtype=f32) * 0.25).astype(f16)
    wd2_np = np.zeros((ND1, 128), dtype=f16)
    wd2_np[:, 0] = np.asarray(d2_w, dtype=f32).reshape(ND1).astype(f16)
    cb_np = np.ascontiguousarray(conv_b.reshape(F, 1), dtype=f32)
    b1_np = np.ascontiguousarray(np.asarray(lc1_b[:NL1], dtype=f32).T)
    b2_np = np.ascontiguousarray(np.asarray(lc2_b, dtype=f32).T)
    db_np = np.ascontiguousarray(d1_b.reshape(ND1, 1), dtype=f32)
    shared = dict(wcp=wcp_np, w1=w1_np, w2=w2_np, wd1=wd1_np, wd2=wd2_np,
                  cb=cb_np, b1=b1_np, b2=b2_np, db=db_np)

    x16 = np.asarray(inputs, dtype=f16).reshape(n_cores, bc * FEAT)
    in_maps = [dict(shared, x=x16[c]) for c in range(n_cores)]
    return in_maps, bias_flags


def kernel(inputs, conv_w, conv_b, lc1_w, lc1_b, lc2_w, lc2_b,
           d1_w, d1_b, d2_w):
    from concourse.bass_utils import run_bass_kernel_spmd

    in_maps, bias_flags = _prep_in_maps(
        inputs, conv_w, conv_b, lc1_w, lc1_b, lc2_w, lc2_b, d1_w, d1_b, d2_w)
    nc = _get_program(NT, bias_flags)
    res = run_bass_kernel_spmd(nc, in_maps, list(range(N_CORES)))
    out = np.concatenate([res.results[c]["y"] for c in range(N_CORES)])
    return out.reshape(B_FULL, 1).astype(np.float32)


# revision 45
# speedup vs baseline: 1.0357x; 1.0141x over previous
"""Trainium2 Bass kernel for nn_BCErrorCNN (dense_cnn).

Network (per sample, input [17, 9]):
  Conv1D(128, k=3, relu) -> [15, 128]   (position 14 dead: never consumed)
  LocallyConnected1D(128, k=3, relu) -> [13, 128]  (position 12 dead)
  MaxPool1D(2) -> [6, 128]
  LocallyConnected1D(128, k=3, relu) -> [4, 128]
  GlobalAvgPool -> [128]; Dense(100, relu); Dense(1, sigmoid)

Sharding: pure data parallelism, batch 32768 -> 8 cores x 4096.
139.5us HW exec vs the 186.4us fp32r baseline; rel err ~2.4e-4.

Fully fp16 datapath (PSUM accumulation stays fp32):
  - X transposed to [feature, batch] by the DMA XBAR straight out of DRAM
    (dma_start(transpose=True), 16-bit only): one [512,128]->[128,512]
    instruction per TA/TB per tile on the otherwise-idle SP engine, so no
    PE transposes, no fp32 input load, and the ACT engine keeps its whole
    budget for PSUM evacuation.
  - conv reads TA/TB directly with zero-padded weights at legal 32-aligned
    base partitions (tile_position rule: K<=32 -> any 32-multiple,
    K<=64 -> {0,64}, else 0) -- no SBUF->SBUF strip DMAs.  Paired,
    row-band-disjoint conv matmuls share a [128,1024] PSUM tile and a
    single strided evacuation, and partially overlap in the PE array.
  - matmuls with <128-wide operands pay ~1.5x: conv contractions are
    zero-padded toward K=64/legal bases, d1's output is padded to 128
    partitions, and d2's stationary free dim is padded to 128 columns.
  - conv pairs + lc1 triples interleaved in PE issue order, plus a
    half-stage software pipeline: each tile's last three lc1 triples, its
    lc2+mean and its dense tail run under the NEXT tile's conv pairs, so
    every serial evac chain has microseconds of PE cover.
  - lc1 evac fused with maxpool: ACT relu-evacs the even position, DVE
    scalar_tensor_tensor computes max(odd+b, relu(even)) which equals
    relu(max(even+b, odd+b)) since relu(x) >= 0.
  - lc2 evac fused with the global-average fold (zero-bias path): two ACT
    relu evacs + two DVE scalar_tensor_tensor accumulations leave the two
    pairwise sums; d1 (wd1 pre-scaled by 1/4) consumes them as two
    accumulating matmuls, so there is no serial mean-reduce chain at all.
  - weights ride the gpsimd software DGE so the hwdge DMA semaphores that
    matmuls wait on count only input transposes; zero biases (the common
    case) are specialized away entirely.
  - per-tile sigmoid straight out of PSUM + per-tile output DMA.
"""

import functools

import numpy as np

# ---- constants (hardcoded per problem spec) --------------------------------
N_CORES = 8
B_FULL = 32768
BC = B_FULL // N_CORES  # per-core batch
NB = 512                # batch tile (columns per matmul)
NT = BC // NB           # batch tiles per core
LIN, CIN, F = 17, 9, 128
FEAT = LIN * CIN        # 153
NPOS = 14               # conv positions actually needed (0..13)
NL1 = 12                # lc1 positions needed (0..11)
NPOOL = 6
NL2 = 4
ND1 = 100

# Conv position p contracts feature rows 9p..9p+26.  TA holds features
# 0..127 on partitions 0..127, TB holds features 25..152.  The matmul
# base-partition rule constrains tile_position[0] by contraction size K:
# K<=32 -> {0,32,64,96}; K<=64 -> {0,64}; else 0.  q0 below is the
# partition where wc row 0 sits (TA: 9p; TB: 9p-25), base is the chosen
# 32-aligned start, K = q0 + 27 - base.
CONV_GEO = [
    # (src, base, q0, K): contraction K is padded to >=64 where the PE
    # would otherwise run small-K matmuls at ~1.5 cyc/col; the extra rows
    # are zeros in the weights and harmless extra terms from TA/TB.
    (0, 0, 0, 64), (0, 0, 9, 36), (0, 0, 18, 64), (0, 0, 27, 64),
    (0, 0, 36, 64), (0, 0, 45, 72), (0, 0, 54, 81), (0, 0, 63, 90),
    (0, 64, 72, 64), (0, 64, 81, 44), (0, 64, 90, 53), (0, 64, 99, 64),
    (1, 64, 83, 64), (1, 64, 92, 55),
]
# Issue order: consecutive matmuls sit in disjoint PE row ranges where
# possible so the systolic array overlaps them.
CONV_ORDER = [0, 8, 1, 9, 2, 10, 5, 11, 4, 12, 3, 13, 6, 7]


def _build_program(nt=NT, bias_flags=(True, True, True, True)):
    conv_bias_zero, lc1_bias_zero, lc2_bias_zero, d1_bias_zero = bias_flags
    import concourse.tile as tile
    from concourse import bacc, mybir

    F32 = mybir.dt.float32
    F16 = mybir.dt.float16
    AF = mybir.ActivationFunctionType
    ALU = mybir.AluOpType

    bc = nt * NB
    nc = bacc.Bacc("TRN2", target_bir_lowering=False, debug=False,
                   num_devices=N_CORES)

    x = nc.dram_tensor("x", [bc * FEAT], F16, kind="ExternalInput").ap()
    wcp = nc.dram_tensor("wcp", [128, NPOS * F], F16, kind="ExternalInput").ap()
    w1 = nc.dram_tensor("w1", [128, NL1 * 3 * F], F16, kind="ExternalInput").ap()
    w2 = nc.dram_tensor("w2", [128, NL2 * 3 * F], F16, kind="ExternalInput").ap()
    wd1 = nc.dram_tensor("wd1", [F, 128], F16, kind="ExternalInput").ap()
    wd2 = nc.dram_tensor("wd2", [ND1, 128], F16, kind="ExternalInput").ap()
    cb = nc.dram_tensor("cb", [F, 1], F32, kind="ExternalInput").ap()
    b1 = nc.dram_tensor("b1", [F, NL1], F32, kind="ExternalInput").ap()
    b2 = nc.dram_tensor("b2", [F, NL2], F32, kind="ExternalInput").ap()
    db = nc.dram_tensor("db", [ND1, 1], F32, kind="ExternalInput").ap()
    y = nc.dram_tensor("y", [bc], F32, kind="ExternalOutput").ap()

    with tile.TileContext(nc) as tc:
        with (
            tc.tile_pool(name="const", bufs=1) as cpool,
            tc.tile_pool(name="t", bufs=2) as tpool,
            tc.tile_pool(name="h", bufs=2) as hpool,
            tc.tile_pool(name="eo", bufs=3) as epool,
            tc.tile_pool(name="m", bufs=2) as mpool,
            tc.tile_pool(name="s2", bufs=2) as s2pool,
            tc.tile_pool(name="ss", bufs=2) as sspool,
            tc.tile_pool(name="s3", bufs=2) as s3pool,
            tc.tile_pool(name="ys", bufs=2) as ypool,
            tc.tile_pool(name="psC", bufs=2, space="PSUM") as psC,
            tc.tile_pool(name="psL", bufs=4, space="PSUM") as psL,
        ):
            def load_T(jt):
                # DMA-XBAR transpose: DRAM [batch, feat] -> SBUF [feat, batch]
                # One [512,128]->[128,512] instruction per target, on sync
                # (tile 0's TB goes on scalar so TA/TB land in parallel).
                TA = tpool.tile([128, NB], F16, tag="TA", name=f"TA{jt}")
                TB = tpool.tile([128, NB], F16, tag="TB", name=f"TB{jt}")
                b0 = jt * NB * FEAT
                srcA = x[b0:b0 + 1].copy()
                srcA.ap = srcA.ap[:0] + [[FEAT, NB], [1, 128]]
                nc.sync.dma_start(TA[:], srcA, transpose=True)
                srcB = x[b0 + 25:b0 + 26].copy()
                srcB.ap = srcB.ap[:0] + [[FEAT, NB], [1, 128]]
                eng = nc.scalar if jt == 0 else nc.sync
                eng.dma_start(TB[:], srcB, transpose=True)
                return TA, TB

            T_pre = {0: load_T(0)}

            # ---- weights (+ any nonzero biases) on the gpsimd software
            # DGE so the hwdge (sync/scalar) DMA semaphores that conv/lc
            # matmuls wait on count only transposes.
            wcpt = cpool.tile([128, NPOS * F], F16)
            nc.gpsimd.dma_start(wcpt[:, :11 * F], wcp[:, :11 * F])
            cbt = b1t = b2t = dbt = None
            if not conv_bias_zero:
                cbt = cpool.tile([F, 1], F32)
                nc.gpsimd.dma_start(cbt[:], cb[:])
            if not lc1_bias_zero:
                b1t = cpool.tile([F, NL1], F32)
                nc.gpsimd.dma_start(b1t[:], b1[:])
            if not lc2_bias_zero:
                b2t = cpool.tile([F, NL2], F32)
                nc.gpsimd.dma_start(b2t[:], b2[:])
            if not d1_bias_zero:
                dbt = cpool.tile([ND1, 1], F32)
                nc.gpsimd.dma_start(dbt[:], db[:])
            nc.gpsimd.dma_start(wcpt[:, 11 * F:], wcp[:, 11 * F:])
            w1t = cpool.tile([128, NL1 * 3 * F], F16)
            nc.gpsimd.dma_start(w1t[:, :NL1 * 3 * F // 2],
                                w1[:, :NL1 * 3 * F // 2])
            nc.gpsimd.dma_start(w1t[:, NL1 * 3 * F // 2:],
                                w1[:, NL1 * 3 * F // 2:])
            w2t = cpool.tile([128, NL2 * 3 * F], F16)
            nc.gpsimd.dma_start(w2t[:], w2[:])
            wd1t = cpool.tile([F, 128], F16)
            nc.gpsimd.dma_start(wd1t[:], wd1[:])
            wd2t = cpool.tile([ND1, 128], F16)
            nc.gpsimd.dma_start(wd2t[:], wd2[:])

            state = {}

            def conv_pair(a, TA, TB, H, it):
                # two row-band-disjoint positions share one [128,1024] psum
                # tile and a single strided evacuation into H.  PSUM half 0
                # always holds the lower position so the H stride stays
                # positive regardless of issue order.
                pi, pj = CONV_ORDER[2 * a], CONV_ORDER[2 * a + 1]
                lo, hi = min(pi, pj), max(pi, pj)
                ps = psC.tile([128, 2 * NB], F32, tag="C", name=f"pC{it}_{a}")
                for p in (pi, pj):
                    src, base, q0, K = CONV_GEO[p]
                    T = TA if src == 0 else TB
                    half = 0 if p == lo else 1
                    nc.tensor.matmul(
                        ps[:, half * NB:(half + 1) * NB],
                        wcpt[base:base + K, p * F:(p + 1) * F],
                        T[base:base + K, :],
                        start=True, stop=True, tile_position=(base, 0))
                hdst = H[:, lo * NB:lo * NB + 1].copy()
                hdst.ap = hdst.ap[:1] + [[(hi - lo) * NB, 2], [1, NB]]
                if a % 2 == 0:
                    if conv_bias_zero:
                        nc.scalar.activation(hdst, ps[:], AF.Relu)
                    else:
                        nc.scalar.activation(hdst, ps[:], AF.Relu,
                                             bias=cbt[:])
                elif conv_bias_zero:
                    nc.vector.tensor_scalar_max(hdst, ps[:], 0.0)
                else:
                    nc.vector.tensor_scalar(hdst, ps[:], cbt[:], 0.0,
                                            op0=ALU.add, op1=ALU.max)

            def lc1_triple(l, H, M, it):
                ps = psL.tile([128, NB], F32, tag="L", name=f"pL{it}_{l}")
                for k in range(3):
                    nc.tensor.matmul(
                        ps[:], w1t[:, (l * 3 + k) * F:(l * 3 + k + 1) * F],
                        H[:, (l + k) * NB:(l + k + 1) * NB],
                        start=(k == 0), stop=(k == 2))
                t = l // 2
                if l % 2 == 0:
                    EO = epool.tile([128, NB], F16, tag="E",
                                    name=f"E{it}_{t}")
                    if lc1_bias_zero:
                        nc.scalar.activation(EO[:], ps[:], AF.Relu)
                    else:
                        nc.scalar.activation(EO[:], ps[:], AF.Relu,
                                             bias=b1t[:, l:l + 1])
                    state[("eo", t)] = EO
                else:
                    EO = state.pop(("eo", t))
                    if lc1_bias_zero:
                        nc.vector.scalar_tensor_tensor(
                            M[:, t * NB:(t + 1) * NB], ps[:], 0.0,
                            EO[:], op0=ALU.max, op1=ALU.max)
                    else:
                        nc.vector.scalar_tensor_tensor(
                            M[:, t * NB:(t + 1) * NB], ps[:],
                            b1t[:, l:l + 1], EO[:],
                            op0=ALU.add, op1=ALU.max)

            def lc2_mean(M, S2, it):
                # lc2 + global-average fold.  Zero-bias path fuses the mean
                # into the evacuations: A_l = relu(ps_l) for l=0,1 (ACT),
                # then DVE scalar_tensor_tensor accumulates relu(ps_{l+2})
                # on top, and one DVE add produces Ssum.
                pss = []
                for l in range(NL2):
                    ps = psL.tile([128, NB], F32, tag="L", name=f"pT{it}_{l}")
                    for k in range(3):
                        nc.tensor.matmul(
                            ps[:], w2t[:, (l * 3 + k) * F:(l * 3 + k + 1) * F],
                            M[:, (l + k) * NB:(l + k + 1) * NB],
                            start=(k == 0), stop=(k == 2))
                    pss.append(ps)
                if lc2_bias_zero:
                    A = sspool.tile([128, 2 * NB], F16, tag="A", name=f"A{it}")
                    for h in range(2):
                        nc.scalar.activation(A[:, h * NB:(h + 1) * NB],
                                             pss[h][:], AF.Relu)
                        nc.vector.scalar_tensor_tensor(
                            S2[:, h * NB:(h + 1) * NB], pss[h + 2][:], 0.0,
                            A[:, h * NB:(h + 1) * NB],
                            op0=ALU.max, op1=ALU.add)
                    state[("s2half", it)] = S2
                else:
                    for l in range(NL2):
                        sdst = S2[:, l * NB:(l + 1) * NB]
                        if l % 2 == 0:
                            nc.scalar.activation(sdst, pss[l][:], AF.Relu,
                                                 bias=b2t[:, l:l + 1])
                        else:
                            nc.vector.tensor_scalar(
                                sdst, pss[l][:], b2t[:, l:l + 1], 0.0,
                                op0=ALU.add, op1=ALU.max)
                    A = sspool.tile([128, 2 * NB], F16, tag="A", name=f"A{it}")
                    nc.gpsimd.tensor_tensor(A[:, 0:NB], S2[:, 0:NB],
                                            S2[:, NB:2 * NB], op=ALU.add)
                    nc.gpsimd.tensor_tensor(A[:, NB:2 * NB],
                                            S2[:, 2 * NB:3 * NB],
                                            S2[:, 3 * NB:4 * NB], op=ALU.add)
                    state[("s2half", it)] = A

            def tail_a(it, h):
                # d1 as two accumulating matmuls over the S2 half-sums
                # (mean folded via wd1 pre-scale): no serial mean chain,
                # each half is consumed as soon as its evac lands.
                if h == 0:
                    pD = psL.tile([128, NB], F32, tag="L", name=f"pD{it}")
                    state[("pd", it)] = pD
                else:
                    pD = state[("pd", it)]
                Shalf = state[("s2half", it)] if h == 0 else \
                    state.pop(("s2half", it))
                nc.tensor.matmul(pD[:, :], wd1t[:],
                                 Shalf[:, h * NB:(h + 1) * NB],
                                 start=(h == 0), stop=(h == 1))

            def tail_b(it):
                pD = state.pop(("pd", it))
                S3 = s3pool.tile([ND1, NB], F16, tag="S3", name=f"S3{it}")
                if d1_bias_zero:
                    nc.scalar.activation(S3[:], pD[0:ND1, :], AF.Relu)
                else:
                    nc.scalar.activation(S3[:], pD[0:ND1, :], AF.Relu,
                                         bias=dbt[:])
                # d2 reuses row 0 of the same bank; the WAR on d1's region is
                # already ordered by the S3 evac that d2 depends on.
                nc.tensor.matmul(pD[:, :], wd2t[:], S3[:],
                                 start=True, stop=True)
                Ys = ypool.tile([1, NB], F32, tag="Y", name=f"Y{it}")
                nc.scalar.activation(Ys[:], pD[0:1, :], AF.Sigmoid)
                nc.sync.dma_start(y[it * NB:(it + 1) * NB], Ys[0:1, :])

            for it in range(nt):
                TA, TB = T_pre.pop(it)
                if it + 1 < nt:
                    T_pre[it + 1] = load_T(it + 1)
                H = hpool.tile([128, NPOS * NB], F16, tag="H", name=f"H{it}")
                M = mpool.tile([128, NPOOL * NB], F16, tag="M", name=f"M{it}")
                S2 = s2pool.tile([128, NL2 * NB], F16, tag="S2",
                                 name=f"S2{it}")

                # conv pairs + lc1 triples interleaved; the previous tile's
                # serial tail chain (mean->d1->relu->d2) hides behind convs.
                # Half-stage software pipeline: the previous tile's last
                # three lc1 triples, its lc2+mean, and its dense tail all
                # run under this tile's conv pairs, so every serial evac
                # chain has multiple us of PE work for cover.
                conv_pair(0, TA, TB, H, it)    # p0, p8
                conv_pair(1, TA, TB, H, it)    # p1, p9
                if it > 0:
                    Hp, Mp, S2p = state.pop(("hms", it - 1))
                    for l in (5, 6, 7):
                        lc1_triple(l, Hp, Mp, it - 1)
                conv_pair(2, TA, TB, H, it)    # p2, p10
                if it > 0:
                    lc2_mean(Mp, S2p, it - 1)
                conv_pair(3, TA, TB, H, it)    # p5, p11
                if it > 0:
                    tail_a(it - 1, 0)
                lc1_triple(0, H, M, it)
                if it > 0:
                    tail_a(it - 1, 1)
                lc1_triple(8, H, M, it)
                if it > 0:
                    tail_b(it - 1)
                conv_pair(4, TA, TB, H, it)    # p4, p12
                lc1_triple(9, H, M, it)
                conv_pair(5, TA, TB, H, it)    # p3, p13
                lc1_triple(10, H, M, it)
                conv_pair(6, TA, TB, H, it)    # p6, p7
                lc1_triple(11, H, M, it)
                for l in (1, 2, 3, 4):
                    lc1_triple(l, H, M, it)
                state[("hms", it)] = (H, M, S2)

            it = nt - 1
            Hp, Mp, S2p = state.pop(("hms", it))
            for l in (5, 6, 7):
                lc1_triple(l, Hp, Mp, it)
            lc2_mean(Mp, S2p, it)
            tail_a(it, 0)
            tail_a(it, 1)
            tail_b(it)

    nc.compile()
    return nc


@functools.lru_cache(maxsize=4)
def _get_program(nt, bias_flags=(True, True, True, True)):
    return _build_program(nt, bias_flags)


def _prep_in_maps(inputs, conv_w, conv_b, lc1_w, lc1_b, lc2_w, lc2_b,
                  d1_w, d1_b, d2_w, nt=NT, n_cores=N_CORES):
    bc = nt * NB
    bias_flags = (not np.any(conv_b), not np.any(lc1_b[:NL1]),
                  not np.any(lc2_b), not np.any(d1_b))
    f16, f32 = np.float16, np.float32

    wc = np.asarray(conv_w, dtype=f32).reshape(27, F)
    wcp_np = np.zeros((128, NPOS * F), dtype=f16)
    for p, (_, _, q0, _) in enumerate(CONV_GEO):
        wcp_np[q0:q0 + 27, p * F:(p + 1) * F] = wc
    w1_np = np.ascontiguousarray(
        np.asarray(lc1_w[:NL1], dtype=f32).reshape(NL1, 3, F, F)
        .transpose(2, 0, 1, 3).reshape(128, NL1 * 3 * F)).astype(f16)
    w2_np = np.ascontiguousarray(
        np.asarray(lc2_w, dtype=f32).reshape(NL2, 3, F, F)
        .transpose(2, 0, 1, 3).reshape(128, NL2 * 3 * F)).astype(f16)
    wd1_np = np.zeros((F, 128), dtype=f16)
    wd1_np[:, :ND1] = (np.asarray(d1_w, dtype=f32) * 0.25).astype(f16)
    wd2_np = np.zeros((ND1, 128), dtype=f16)
    wd2_np[:, 0] = np.asarray(d2_w, dtype=f32).reshape(ND1).astype(f16)
    cb_np = np.ascontiguousarray(conv_b.reshape(F, 1), dtype=f32)
    b1_np = np.ascontiguousarray(np.asarray(lc1_b[:NL1], dtype=f32).T)
    b2_np = np.ascontiguousarray(np.asarray(lc2_b, dtype=f32).T)
    db_np = np.ascontiguousarray(d1_b.reshape(ND1, 1), dtype=f32)
    shared = dict(wcp=wcp_np, w1=w1_np, w2=w2_np, wd1=wd1_np, wd2=wd2_np,
                  cb=cb_np, b1=b1_np, b2=b2_np, db=db_np)

    x16 = np.asarray(inputs, dtype=f16).reshape(n_cores, bc * FEAT)
    in_maps = [dict(shared, x=x16[c]) for c in range(n_cores)]
    return in_maps, bias_flags


def kernel(inputs, conv_w, conv_b, lc1_w, lc1_b, lc2_w, lc2_b,
           d1_w, d1_b, d2_w):
    from concourse.bass_utils import run_bass_kernel_spmd

    in_maps, bias_flags = _prep_in_maps(
        inputs, conv_w, conv_b, lc1_w, lc1_b, lc2_w, lc2_b, d1_w, d1_b, d2_w)
    nc = _get_program(NT, bias_flags)
    res = run_bass_kernel_spmd(nc, in_maps, list(range(N_CORES)))
    out = np.concatenate([res.results[c]["y"] for c in range(N_CORES)])
    return out.reshape(B_FULL, 1).astype(np.float32)
